# revision 1
# baseline (speedup 1.0000x reference)
"""Trainium2 Bass kernel for nn_B2GravNetBlock (GravNet message-passing block).

Contract: kernel(**inputs) takes FULL inputs (as produced by the problem's
setup_inputs) and returns the FULL [131072, 128] float32 output. Internally
the 128 graphs are sharded 16-per-core across 8 NeuronCores (pure data
parallel, weights replicated), one Bass SPMD program.

Per-core pipeline (16 graphs x 1024 nodes = 16384 nodes):
  phase 1 (node-parallel, feature-major):
    xT  <- PE-transpose of x tiles
    r1T = relu(W1.T @ xT + b1)          (BN1 folded into W2/b2)
    r2T = relu(W2f.T @ r1T + b2f)       (BN2 folded into W3/b3)
    hT  = W3f.T @ r2T + b3f  -> spilled to DRAM (feature-major)
    dops[:, 0, :] = A = [s; sq; 1], dops[:, 1, :] = B = [2s; -1; -sq]
    f   = h @ Wh + bh  (node-major, bias via rank-1 matmul) -> DRAM
  phase 2 (per graph g, per 128-row tile):
    nd2 = A.T @ B in PSUM  (nd2[i,j] = -|s_i - s_j|^2, exact f32 bilinear)
    max8 -> top-8 values desc; max_index -> their indices
    w = exp(10 * min(v, 0)); indirect-DMA gather of f rows (top-5)
    msg = fnb * w ; mean = sum_k ; max = max_k   (mean /5 folded into Wo2)
    aggT = PE-transpose([mean || max])
    out = h @ Wo1f + aggT.T @ [Wo2m; Wo2x] + bof  (BN3 folded in)
"""

import sys

import numpy as np

if "/opt/trn_rl_repo" not in sys.path:
    sys.path.insert(0, "/opt/trn_rl_repo")

# ---- problem constants (hardcoded per contract) ----
B, NPG = 128, 1024
IN, HID, OUT = 16, 128, 128
S, P, K = 16, 64, 5
EPS = 1e-5
N_CORES = 8
G_PER_CORE = B // N_CORES          # 16 graphs per core
NPC = G_PER_CORE * NPG             # 16384 nodes per core
CHUNK = 512                        # phase-1 free-dim chunk


def _fold_weights(inp):
    """Host-side BN folding; returns dict of small replicated arrays."""
    f32 = np.float32
    g = {k: np.asarray(v, dtype=f32) for k, v in inp.items()}
    a1 = g["g1"] / np.sqrt(g["v1"] + EPS)
    c1 = g["be1"] - g["m1"] * a1
    a2 = g["g2"] / np.sqrt(g["v2"] + EPS)
    c2 = g["be2"] - g["m2"] * a2
    a3 = g["g3"] / np.sqrt(g["v3"] + EPS)
    c3 = g["be3"] - g["m3"] * a3

    W2f = (a1[:, None] * g["W2"]).astype(f32)
    b2f = (g["b2"] + c1 @ g["W2"]).astype(f32)
    W3f = (a2[:, None] * g["W3"]).astype(f32)
    b3f = (g["b3"] + c2 @ g["W3"]).astype(f32)
    Wo1f = (g["Wo1"] * a3[None, :]).astype(f32)
    Wo2f = (g["Wo2"] * a3[None, :]).astype(f32)
    Wo2m = (Wo2f[:P] / 5.0).astype(f32)          # mean part, /K folded
    Wo2x = Wo2f[P:].astype(f32)                  # max part
    bof = (g["bo2"] * a3 + c3).astype(f32)

    bsf = np.concatenate([g["bs"], 2.0 * g["bs"]]).astype(f32)
    # stacked agg output weight [2P, OUT]
    Wo2mx = np.concatenate([Wo2m, Wo2x], axis=0).astype(f32)

    return dict(
        W1=g["W1"], b1=g["b1"].reshape(HID, 1),
        W2f=W2f, b2f=b2f.reshape(HID, 1),
        W3f=W3f, b3f=b3f.reshape(HID, 1),
        Ws=g["Ws"], bs1=bsf[:S].reshape(S, 1), bs2=bsf[S:].reshape(S, 1),
        Wh=g["Wh"], bh_row=g["bh"].reshape(1, P),
        Wo1f=Wo1f, Wo2mx=Wo2mx, bof_row=bof.reshape(1, OUT),
    )


WEIGHT_SPECS = [
    ("W1", [IN, HID]), ("b1", [HID, 1]),
    ("W2f", [HID, HID]), ("b2f", [HID, 1]),
    ("W3f", [HID, HID]), ("b3f", [HID, 1]),
    ("Ws", [HID, S]), ("bs1", [S, 1]), ("bs2", [S, 1]),
    ("Wh", [HID, P]), ("bh_row", [1, P]),
    ("Wo1f", [HID, OUT]), ("Wo2mx", [2 * P, OUT]), ("bof_row", [1, OUT]),
]


def build_nc(n_graphs=G_PER_CORE):
    """Builds the single-core Bass program (SPMD: same program, 8 cores)."""
    from contextlib import ExitStack

    import concourse.bass as bass
    import concourse.bacc as bacc
    import concourse.mybir as mybir
    import concourse.tile as tile
    from concourse.masks import make_identity

    f32 = mybir.dt.float32
    u32 = mybir.dt.uint32
    u16 = mybir.dt.uint16
    i16 = mybir.dt.int16
    AF = mybir.ActivationFunctionType
    ALU = mybir.AluOpType
    AX = mybir.AxisListType

    npc = n_graphs * NPG
    n_chunks = npc // CHUNK
    TPG = NPG // 128               # 8 i-tiles per graph
    NIG = NPG * 5                  # gathered rows per graph

    nc = bacc.Bacc(debug=False)

    # ---- DRAM I/O ----
    x_d = nc.dram_tensor("xT", [IN, npc], f32, kind="ExternalInput")
    w_d = {
        name: nc.dram_tensor(name, shape, f32, kind="ExternalInput")
        for name, shape in WEIGHT_SPECS
    }
    out_d = nc.dram_tensor("out", [npc, OUT], f32, kind="ExternalOutput")
    f_d = nc.dram_tensor("f_scratch", [npc, P], f32)    # gather source
    ib_d = nc.dram_tensor("idx_scratch", [n_graphs, 128, 5 * TPG], i16)
    h_d = nc.dram_tensor("h_scratch", [HID, npc], f32)  # feature-major h

    with ExitStack() as ctx:
        tc = ctx.enter_context(tile.TileContext(nc))

        # ---- constants ----
        const = ctx.enter_context(tc.tile_pool(name="const", bufs=1))
        w_sb = {}
        for name, shape in WEIGHT_SPECS:
            t = const.tile(shape, f32, tag=f"w_{name}")
            nc.sync.dma_start(out=t[:], in_=w_d[name][:])
            w_sb[name] = t
        ident = const.tile([128, 128], f32, tag="ident")
        make_identity(nc, ident[:])
        # ones row for rank-1 bias matmuls
        ones1 = const.tile([1, 128], f32, tag="ones1")
        nc.vector.memset(ones1[:], 1.0)

        # ---- persistent per-core tensors ----
        big = ctx.enter_context(tc.tile_pool(name="big", bufs=1))
        # Distance-matmul operands, both base-partition 0, K=48:
        #   A = dops[:, 0, :] = [-1 x16 | -1 x16 | s ]   (rows 0:32 = -1)
        #   B = dops[:, 1, :] = [ s^2   |  0 x16 | 2s]
        # A.T @ B gives nd2'[i,j] = 2 s_i.s_j - |s_j|^2  (= nd2 + |s_i|^2).
        # The per-row constant |s_i|^2 is exactly the top-1 of row i (self),
        # so after max8 we correct v = min(v' - v'[0], 0).
        dops = big.tile([48, 2, npc], f32, tag="dops")
        nc.gpsimd.memset(dops[0:32, 0, :], -1.0)
        nc.gpsimd.memset(dops[0:32, 1, :], 0.0)

        # ================= phase 1: MLP over all nodes =================
        with tc.tile_pool(name="p1_sbuf", bufs=3) as sp, \
             tc.tile_pool(name="p1_psum", bufs=3, space="PSUM") as pp, \
             tc.tile_pool(name="p1_psum_small", bufs=2, space="PSUM") as pps:
            for c in range(n_chunks):
                lo = c * CHUNK
                sl = slice(lo, lo + CHUNK)
                # xT chunk [IN, 512] straight from DRAM (pre-transposed on host)
                xT = sp.tile([IN, CHUNK], f32, tag="xT")
                nc.sync.dma_start(out=xT[:], in_=x_d[:, sl])

                # L1: r1 = relu(W1.T @ xT + b1)
                ps1 = pp.tile([HID, CHUNK], f32, tag="mlp")
                nc.tensor.matmul(
                    out=ps1[:], lhsT=w_sb["W1"][:],
                    rhs=xT[:], start=True, stop=True)
                r1 = sp.tile([HID, CHUNK], f32, tag="r1")
                nc.scalar.activation(r1[:], ps1[:], AF.Relu, bias=w_sb["b1"][:])

                # L2
                ps2 = pp.tile([HID, CHUNK], f32, tag="mlp")
                nc.tensor.matmul(
                    out=ps2[:], lhsT=w_sb["W2f"][:],
                    rhs=r1[:], start=True, stop=True)
                r2 = sp.tile([HID, CHUNK], f32, tag="r2")
                nc.scalar.activation(r2[:], ps2[:], AF.Relu, bias=w_sb["b2f"][:])

                # L3 -> h chunk (feature-major), spilled to DRAM
                ps3 = pp.tile([HID, CHUNK], f32, tag="mlp")
                nc.tensor.matmul(
                    out=ps3[:], lhsT=w_sb["W3f"][:],
                    rhs=r2[:], start=True, stop=True)
                hch = sp.tile([HID, CHUNK], f32, tag="hch")
                nc.scalar.activation(hch[:], ps3[:], AF.Identity,
                                     bias=w_sb["b3f"][:])
                nc.sync.dma_start(out=h_d[:, sl], in_=hch[:])

                # s rows at partitions 32:48 (A), 2s = 2*ps+2bs (B),
                # s^2 at B rows 0:16
                ps4 = pps.tile([S, CHUNK], f32, tag="small")
                nc.tensor.matmul(
                    out=ps4[:], lhsT=w_sb["Ws"][:],
                    rhs=hch[:], start=True, stop=True)
                nc.scalar.activation(dops[32:48, 0, sl], ps4[:], AF.Identity,
                                     bias=w_sb["bs1"][:])
                nc.scalar.activation(dops[32:48, 1, sl], ps4[:],
                                     AF.Identity, bias=w_sb["bs2"][:],
                                     scale=2.0)
                nc.scalar.square(dops[0:S, 1, sl], dops[32:48, 0, sl])

                # f natural [node, P] -> DRAM (bias via rank-1 matmul on ones)
                fsb = sp.tile([128, 4, P], f32, tag="fsb")
                for t in range(4):
                    psf = pps.tile([128, P], f32, tag="small")
                    nc.tensor.matmul(out=psf[:],
                                     lhsT=hch[:, t * 128:(t + 1) * 128],
                                     rhs=w_sb["Wh"][:],
                                     start=True, stop=False)
                    nc.tensor.matmul(out=psf[:], lhsT=ones1[:],
                                     rhs=w_sb["bh_row"][:],
                                     start=False, stop=True)
                    nc.scalar.copy(out=fsb[:, t], in_=psf[:])
                nc.sync.dma_start(
                    out=f_d[lo:lo + CHUNK, :].rearrange(
                        "(t p) c -> p t c", p=128), in_=fsb[:])

        # ================= phase 2: per-graph kNN + aggregation ============
        with tc.tile_pool(name="p2_sbuf", bufs=2) as sp, \
             tc.tile_pool(name="p2_small", bufs=3) as sps, \
             tc.tile_pool(name="p2_nd2", bufs=2, space="PSUM") as pnd, \
             tc.tile_pool(name="p2_psum", bufs=2, space="PSUM") as pp:
            for g in range(n_graphs):
                gbase = g * NPG
                # per-graph state
                v_g = sp.tile([128, TPG, 8], f32, tag="v_g")
                vc_g = sp.tile([128, TPG, 8], f32, tag="vc_g")
                idx_g = sp.tile([128, TPG, 8], u16, tag="idx_g")
                fnb_g = sp.tile([128, TPG, K, P], f32, tag="fnb_g")
                w_g = sp.tile([128, TPG, 8], f32, tag="w_g")
                agg = sp.tile([128, TPG, 2 * P], f32, tag="agg")

                for t in range(TPG):
                    i0 = gbase + t * 128
                    nd2 = pnd.tile([128, NPG], f32, tag="nd2")
                    for jh in range(2):
                        nc.tensor.matmul(
                            out=nd2[:, jh * 512:(jh + 1) * 512],
                            lhsT=dops[:, 0, i0:i0 + 128],
                            rhs=dops[:, 1,
                                     gbase + jh * 512:gbase + (jh + 1) * 512
                                     ],
                            start=True, stop=True)
                    nc.vector.max(out=v_g[:, t], in_=nd2[:])
                    nc.vector.max_index(out=idx_g[:, t], in_max=v_g[:, t],
                                        in_values=nd2[:])
                    # v = min(v' - v'[0], 0): subtract |s_i|^2, clamp d2 >= 0
                    nc.vector.tensor_scalar(
                        out=vc_g[:, t], in0=v_g[:, t],
                        scalar1=v_g[:, t, 0:1], scalar2=0.0,
                        op0=ALU.subtract, op1=ALU.min)

                # weights: w = exp(10 * v)
                nc.scalar.activation(
                    w_g[:].rearrange("p t k -> p (t k)"),
                    vc_g[:].rearrange("p t k -> p (t k)"),
                    AF.Exp, scale=10.0)

                # idx shuffle to dma_gather wrapped layout via DRAM:
                # hop1: buf[q, s=(t*5+k)] = idx_g[q, t, k]
                nc.sync.dma_start(
                    out=ib_d[g].rearrange("q (t k) -> q t k", t=TPG, k=K),
                    in_=idx_g[:, :, 0:K].bitcast(i16))
                # hop2: idxw[16g2+pl, 8*s+fq] = buf[16*fq+pl, s], all groups
                idxw = sp.tile([128, NIG // 16], i16, tag="idxw")
                ibv = ib_d[g].rearrange("(fq pl) s -> pl s fq", pl=16)
                for g2 in range(8):
                    nc.sync.dma_start(
                        out=idxw[16 * g2:16 * (g2 + 1), :].rearrange(
                            "pl (s fq) -> pl s fq", fq=8),
                        in_=ibv)
                # bulk gather: fnb[q, s, :] = f[g*NPG + idx_flat[s*128+q], :]
                # (num_idxs > 640 overflows the SWDGE ring -> per-tile chunks)
                fnbv = fnb_g[:].rearrange("p t k c -> p (t k) c")
                for t in range(TPG):
                    nc.gpsimd.dma_gather(
                        out_ap=fnbv[:, 5 * t:5 * (t + 1), :],
                        in_ap=f_d[gbase:gbase + NPG, :],
                        idxs_ap=idxw[:, 40 * t:40 * (t + 1)],
                        num_idxs=640, num_idxs_reg=640, elem_size=P)

                # msg = fnb * w
                msg = sp.tile([128, TPG, K, P], f32, tag="msg")
                nc.vector.tensor_tensor(
                    out=msg[:], in0=fnb_g[:],
                    in1=w_g[:, :, 0:K].to_broadcast([128, TPG, K, P]),
                    op=ALU.mult)
                # mean (sum; /5 folded into Wo2m) and max over k
                mview = msg[:].rearrange("p t k c -> p t c k")
                nc.vector.tensor_reduce(
                    out=agg[:, :, 0:P], in_=mview, axis=AX.X, op=ALU.add)
                nc.vector.tensor_reduce(
                    out=agg[:, :, P:2 * P], in_=mview, axis=AX.X, op=ALU.max)

                # per-tile: transpose agg, output matmuls; batched h load
                # and one batched out store per graph
                hg = sp.tile([HID, NPG], f32, tag="hg")
                nc.sync.dma_start(out=hg[:], in_=h_d[:, gbase:gbase + NPG])
                osb = sp.tile([128, TPG, OUT], f32, tag="osb")
                for t in range(TPG):
                    ps_at = pp.tile([2 * P, 128], f32, tag="ps_at")
                    nc.tensor.transpose(out=ps_at[:], in_=agg[:, t],
                                        identity=ident[:])
                    aggT = sps.tile([2 * P, 128], f32, tag="aggT")
                    nc.scalar.copy(out=aggT[:], in_=ps_at[:])

                    pso = pp.tile([128, OUT], f32, tag="pso")
                    nc.tensor.matmul(out=pso[:],
                                     lhsT=hg[:, t * 128:(t + 1) * 128],
                                     rhs=w_sb["Wo1f"][:], start=True,
                                     stop=False)
                    nc.tensor.matmul(out=pso[:], lhsT=aggT[:],
                                     rhs=w_sb["Wo2mx"][:], start=False,
                                     stop=False)
                    nc.tensor.matmul(out=pso[:], lhsT=ones1[:],
                                     rhs=w_sb["bof_row"][:],
                                     start=False, stop=True)
                    nc.scalar.copy(out=osb[:, t], in_=pso[:])
                nc.sync.dma_start(
                    out=out_d[gbase:gbase + NPG, :].rearrange(
                        "(t p) c -> p t c", p=128), in_=osb[:])

    nc.compile()
    return nc


_BUILD_CACHE = {}


def _get_nc(n_graphs=G_PER_CORE):
    if n_graphs not in _BUILD_CACHE:
        _BUILD_CACHE[n_graphs] = build_nc(n_graphs)
    return _BUILD_CACHE[n_graphs]


def kernel_raw(x, inp, trace=False):
    from concourse.bass_utils import run_bass_kernel_spmd

    x = np.ascontiguousarray(np.asarray(x, dtype=np.float32))
    folded = _fold_weights(inp)
    nc = _get_nc()

    in_maps = []
    for c in range(N_CORES):
        m = {name: folded[name] for name, _ in WEIGHT_SPECS}
        m["xT"] = np.ascontiguousarray(x[c * NPC:(c + 1) * NPC].T)
        in_maps.append(m)

    res = run_bass_kernel_spmd(nc, in_maps, list(range(N_CORES)), trace=trace)
    out = np.concatenate([r["out"] for r in res.results], axis=0)
    return out, res


def kernel(x, batch=None, **inp):
    return kernel_raw(x, inp)[0]


if __name__ == "__main__":
    nc = build_nc(int(sys.argv[1]) if len(sys.argv) > 1 else 1)
    print("built ok")

